# revision 15
# baseline (speedup 1.0000x reference)
"""Trainium2 Bass kernel for nn_MultiHeadAttention (B=4, S=2048, D=1024, H=16,
causal mask) on 8 NeuronCores.

Sharding: core c handles batch b = c//2 and heads hs = (c%2)*8 .. hs+8
(data parallel on B, tensor parallel on heads). Host sums the two per-batch
partial o-projections (the "all-reduce") and adds the bias.

Device kernel per core (SPMD, identical program, different data):
  Phase A: QK^T/V projections from host-transposed x^T and sliced weights.
  Phase B: per head: dual-orientation attention:
    [q,k]: scores -> (+causal -1e9 on diag blocks) -> exp with accum_out
           rowsum -> reciprocal -> normalize -> DMA attn out (upper-triangle
           blocks skipped; output buffers are pre-zeroed by the runtime).
    [k,q]: scores^T -> masked -> exp -> attn^T @ V accumulation (av^T),
           normalized by broadcast 1/rowsum, stacked for the o-projection.
  Phase C: o_partial = av_all @ w_o_slice^T -> DMA out.
"""
import sys
import os
import math

sys.path.insert(0, "/opt/trn_rl_repo")
import numpy as np

B, S, D, H = 4, 2048, 1024, 16
DK = D // H          # 64
HPC = H // 2         # 8 heads per core
N_CORES = 8
NEG = -1.0e9

_prog_cache = {}


def _build(causal: bool, reps: int = 1, dma_eng: str = "sync"):
    import concourse.bass as bass  # noqa: F401
    import concourse.mybir as mybir
    import concourse.tile as tile
    from concourse import bacc

    f32 = mybir.dt.float32
    f32r = mybir.dt.float32r
    bf16 = mybir.dt.bfloat16
    AF = mybir.ActivationFunctionType

    nc = bacc.Bacc("TRN2", target_bir_lowering=False, debug=False,
                   num_devices=N_CORES)

    # ---- I/O ----
    xqT = nc.dram_tensor("xqT", [D, S], f32r, kind="ExternalInput")
    xkT = nc.dram_tensor("xkT", [D, S], f32r, kind="ExternalInput")
    xvT = nc.dram_tensor("xvT", [D, S], f32r, kind="ExternalInput")
    wqT = nc.dram_tensor("wqT", [D, 512], f32r, kind="ExternalInput")
    wkT = nc.dram_tensor("wkT", [D, 512], f32r, kind="ExternalInput")
    wvT = nc.dram_tensor("wvT", [D, 512], f32r, kind="ExternalInput")
    woT = nc.dram_tensor("woT", [512, D], bf16, kind="ExternalInput")
    # causal consts: madd_d [128,128] strict-upper -1e9 (for [q,k] diag
    # blocks); madd_tw [128,1024] with madd_tw[kp,u] = -1e9 if u < kp+512
    # (sliced per diagonal chunk in the [k,q] orientation).
    madd_d = nc.dram_tensor("madd_d", [128, 128], f32, kind="ExternalInput")
    madd_tw = nc.dram_tensor("madd_tw", [128, 1024], f32, kind="ExternalInput")
    if not causal:
        maddf = nc.dram_tensor("maddf", [S, S], f32, kind="ExternalInput")
        maddfT = nc.dram_tensor("maddfT", [S, S], f32, kind="ExternalInput")

    attn = nc.dram_tensor("attn", [HPC, S, S], f32, kind="ExternalOutput")
    o = nc.dram_tensor("o", [S, D], f32, kind="ExternalOutput")

    dma = nc.sync if dma_eng == "sync" else nc.gpsimd

    NQT = S // 128       # 16 q tiles
    NQC = S // 512       # 4 q chunks
    NKC = S // 128       # 16 k chunks of 128

    with tile.TileContext(nc) as tc:
        import contextlib
        ctx = contextlib.ExitStack()
        with ctx:
            res = ctx.enter_context(tc.tile_pool(name="resident", bufs=1))
            # persistent tiles
            QT = [res.tile([128, S], f32r, tag=f"qt{i}", name=f"qt{i}") for i in range(4)]
            KT = [res.tile([128, S], f32r, tag=f"kt{i}", name=f"kt{i}") for i in range(4)]
            V = [res.tile([128, 512], bf16, tag=f"v{i}", name=f"v{i}") for i in range(16)]
            AVT = [res.tile([128, S], bf16, tag=f"avt{i}", name=f"avt{i}") for i in range(4)]
            WO = [res.tile([128, D], bf16, tag=f"wo{i}", name=f"wo{i}") for i in range(4)]
            MD = res.tile([128, 128], f32, tag="md", name="md")
            MTW = res.tile([128, 1024], f32, tag="mtw", name="mtw")
            warm = res.tile([128, 1], f32, tag="warm", name="warm")
            nc.gpsimd.memset(warm[:], 0.0)
            nc.scalar.activation(warm[:], warm[:], AF.Exp, scale=1.0)
            dma.dma_start(MD[:], madd_d[:, :])
            dma.dma_start(MTW[:], madd_tw[:, :])
            for i in range(4):
                dma.dma_start(WO[i][:], woT[i * 128:(i + 1) * 128, :])

            loop_cm = tc.For_i(0, reps, 1) if reps > 1 else contextlib.nullcontext()
            with loop_cm:
                bctx = contextlib.ExitStack()
                with bctx:
                    # --- phase-B pools that must coexist with phase A ---
                    ps_s = bctx.enter_context(
                        tc.tile_pool(name="ps_s", bufs=2, space="PSUM"))
                    expp = bctx.enter_context(tc.tile_pool(name="expp", bufs=3))
                    expt = bctx.enter_context(tc.tile_pool(name="expt", bufs=4))
                    small = bctx.enter_context(tc.tile_pool(name="small", bufs=8))
                    rqp = bctx.enter_context(tc.tile_pool(name="rqp", bufs=34))
                    rbp = bctx.enter_context(tc.tile_pool(name="rbp", bufs=4))
                    mfp = bctx.enter_context(tc.tile_pool(name="mfp", bufs=3))
                    osb = bctx.enter_context(tc.tile_pool(name="osb", bufs=2))
                    rdram = bctx.enter_context(
                        tc.tile_pool(name="rdram", bufs=6, space="DRAM"))

                    rq_of = {}

                    def emit_proj(xT, wT, kind, xpool, wpool, pproj,
                                  weave=None):
                        wt = []
                        for c in range(8):
                            w = wpool.tile([128, 512], f32r, tag="win", name="win")
                            dma.dma_start(w[:], wT[c * 128:(c + 1) * 128, :])
                            wt.append(w)
                        for sh in range(2):
                            s0 = sh * 1024
                            xt = []
                            for c in range(8):
                                t = xpool.tile([128, 1024], f32r, tag="xin", name="xin")
                                dma.dma_start(
                                    t[:], xT[c * 128:(c + 1) * 128, s0:s0 + 1024])
                                xt.append(t)
                            if kind in ("q", "k"):
                                dst = QT if kind == "q" else KT
                                for hp in range(4):
                                    for scl in range(2):
                                        sc = sh * 2 + scl
                                        ps = pproj.tile([128, 512], f32,
                                                        tag="psp", name="psp")
                                        for c in range(8):
                                            nc.tensor.matmul(
                                                ps[:],
                                                lhsT=wt[c][:, hp * 128:(hp + 1) * 128],
                                                rhs=xt[c][:, scl * 512:(scl + 1) * 512],
                                                start=(c == 0), stop=(c == 7))
                                        nc.vector.tensor_copy(
                                            dst[hp][:, sc * 512:(sc + 1) * 512], ps[:])
                                    if weave is not None:
                                        weave(sh, hp)
                            else:
                                for stl in range(8):
                                    st = sh * 8 + stl
                                    ps = pproj.tile([128, 512], f32,
                                                    tag="psp", name="psp")
                                    for c in range(8):
                                        nc.tensor.matmul(
                                            ps[:],
                                            lhsT=xt[c][:, stl * 128:(stl + 1) * 128],
                                            rhs=wt[c][:],
                                            start=(c == 0), stop=(c == 7))
                                    nc.vector.tensor_copy(V[st][:], ps[:])

                    def emit_qk(qc, hl):
                        # [q,k] orientation for q-chunk qc, one head
                        if True:
                            hp, ho = hl // 2, (hl % 2) * 64
                            rq = rqp.tile([128, 4], f32, tag="rq", name="rq")
                            rq_of[(qc, hl)] = rq
                            for qj in range(4):
                                qi = qc * 4 + qj
                                live = (qi + 1) * 128 if causal else S
                                nch = (live + 1023) // 1024
                                ex = expp.tile([128, S], f32, tag="exp", name="exp")
                                accs = []
                                for ci in range(nch):
                                    k0 = ci * 1024
                                    k1 = min(live, k0 + 1024)
                                    ps = ps_s.tile([128, 1024], f32,
                                                   tag="pss", name="pss")
                                    for kc0 in range(k0, k1, 512):
                                        w = min(512, k1 - kc0)
                                        nc.tensor.matmul(
                                            ps[:, kc0 - k0:kc0 - k0 + w],
                                            lhsT=QT[hp][ho:ho + 64,
                                                        qi * 128:(qi + 1) * 128],
                                            rhs=KT[hp][ho:ho + 64, kc0:kc0 + w],
                                            start=True, stop=True)
                                    if causal and k0 <= qi * 128 < k1:
                                        off = qi * 128 - k0
                                        nc.vector.tensor_add(
                                            ps[:, off:off + 128],
                                            ps[:, off:off + 128], MD[:])
                                    if not causal:
                                        mf = mfp.tile([128, 1024], f32,
                                                      tag="mf", name="mf")
                                        dma.dma_start(
                                            mf[:, :k1 - k0],
                                            maddf[qi * 128:(qi + 1) * 128, k0:k1])
                                        nc.vector.tensor_add(
                                            ps[:, :k1 - k0], ps[:, :k1 - k0],
                                            mf[:, :k1 - k0])
                                    acc = small.tile([128, 1], f32,
                                                     tag="acc", name="acc")
                                    nc.scalar.activation(
                                        ex[:, k0:k1], ps[:, :k1 - k0], AF.Exp,
                                        scale=0.125, accum_out=acc[:])
                                    accs.append(acc)
                                tot = accs[0]
                                if len(accs) == 2:
                                    tot = small.tile([128, 1], f32,
                                                     tag="acc", name="acc")
                                    nc.vector.tensor_add(
                                        tot[:], accs[0][:], accs[1][:])
                                nc.vector.reciprocal(rq[:, qj:qj + 1], tot[:])
                                nc.vector.tensor_scalar_mul(
                                    ex[:, :live], ex[:, :live], rq[:, qj:qj + 1])
                                dma.dma_start(
                                    attn[hl, qi * 128:(qi + 1) * 128, 0:live],
                                    ex[:, :live])

                    def emit_kq(qc, hl, ps_t, ps_av):
                        # [k,q] orientation + attn^T @ V for q-chunk qc, one head
                        if True:
                            hp, ho = hl // 2, (hl % 2) * 64
                            rq = rq_of.pop((qc, hl))
                            rs = rdram.tile([512], f32, tag="rs", name="rs")
                            nc.gpsimd.dma_start(
                                rs.rearrange("(t p) -> p t", p=128), rq[:])
                            rb = rbp.tile([64, 512], f32, tag="rb", name="rb")
                            nc.gpsimd.dma_start(
                                rb[:], rs[None, :].broadcast_to([64, 512]))
                            av = ps_av.tile([64, 512], f32, tag="av", name="av")
                            kmax = (qc + 1) * 4 if causal else NKC
                            for kc in range(kmax):
                                # diagonal chunks: only q-range [d, 512) is live
                                d = max(0, 128 * kc - 512 * qc) if causal else 0
                                w = 512 - d
                                pst = ps_t.tile([128, 512], f32,
                                                tag="pst", name="pst")
                                nc.tensor.matmul(
                                    pst[:, :w],
                                    lhsT=KT[hp][ho:ho + 64,
                                                kc * 128:(kc + 1) * 128],
                                    rhs=QT[hp][ho:ho + 64,
                                               qc * 512 + d:(qc + 1) * 512],
                                    start=True, stop=True)
                                if causal and kc >= 4 * qc:
                                    nc.vector.tensor_add(
                                        pst[:, :w], pst[:, :w],
                                        MTW[:, 512:1024 - d])
                                if not causal:
                                    mf = mfp.tile([128, 1024], f32,
                                                  tag="mf", name="mf")
                                    dma.dma_start(
                                        mf[:, :512],
                                        maddfT[kc * 128:(kc + 1) * 128,
                                               qc * 512:(qc + 1) * 512])
                                    nc.vector.tensor_add(
                                        pst[:], pst[:], mf[:, :512])
                                et = expt.tile([128, 512], bf16,
                                               tag="expT", name="expT")
                                nc.scalar.activation(
                                    et[:, :w], pst[:, :w], AF.Exp, scale=0.125)
                                nc.tensor.matmul(
                                    av[:, d:512],
                                    lhsT=V[kc][:, hl * 64:hl * 64 + 64],
                                    rhs=et[:, :w],
                                    start=(kc == 0), stop=(kc == kmax - 1))
                            nc.vector.tensor_mul(
                                AVT[hp][ho:ho + 64, qc * 512:(qc + 1) * 512],
                                av[:], rb[:])

                    def emit_o_tile(qt, ps_t):
                        if True:
                            
                            ot = osb.tile([128, D], f32, tag="ot", name="ot")
                            for nh in range(2):
                                po = ps_t.tile([128, 512], f32,
                                               tag="pst", name="pst")
                                for cc in range(4):
                                    nc.tensor.matmul(
                                        po[:],
                                        lhsT=AVT[cc][:, qt * 128:(qt + 1) * 128],
                                        rhs=WO[cc][:, nh * 512:(nh + 1) * 512],
                                        start=(cc == 0), stop=(cc == 3))
                                nc.vector.tensor_copy(
                                    ot[:, nh * 512:(nh + 1) * 512], po[:])
                            dma.dma_start(o[qt * 128:(qt + 1) * 128, :], ot[:])

                    # ---- pipelined emission ----
                    actx = contextlib.ExitStack()
                    with actx:
                        xpool = actx.enter_context(tc.tile_pool(name="xin", bufs=8))
                        wpool = actx.enter_context(tc.tile_pool(name="win", bufs=8))
                        pproj = actx.enter_context(
                            tc.tile_pool(name="ps_proj", bufs=2, space="PSUM"))
                        emit_proj(xkT, wkT, "k", xpool, wpool, pproj)

                        def _qweave(sh, hp):
                            if sh == 0:
                                for h in (2 * hp, 2 * hp + 1):
                                    emit_qk(0, h)
                                    emit_qk(1, h)
                            else:
                                for h in (2 * hp, 2 * hp + 1):
                                    emit_qk(2, h)
                        emit_proj(xqT, wqT, "q", xpool, wpool, pproj,
                                  weave=_qweave)
                        emit_proj(xvT, wvT, "v", xpool, wpool, pproj)
                    ps_t = bctx.enter_context(
                        tc.tile_pool(name="ps_t", bufs=3, space="PSUM"))
                    ps_av = bctx.enter_context(
                        tc.tile_pool(name="ps_av", bufs=1, space="PSUM"))
                    for hl in range(HPC):
                        emit_kq(0, hl, ps_t, ps_av)
                        emit_kq(1, hl, ps_t, ps_av)
                        emit_qk(3, hl)
                    for hl in range(HPC):
                        emit_kq(2, hl, ps_t, ps_av)
                        emit_o_tile(hl, ps_t)
                    for hl in range(HPC):
                        emit_kq(3, hl, ps_t, ps_av)
                        emit_o_tile(8 + hl, ps_t)
                    for qt in (12, 13, 14, 15):
                        emit_o_tile(qt, ps_t)
    nc.compile()
    return nc


def _get_prog(causal: bool, reps: int = 1):
    key = (causal, reps)
    if key not in _prog_cache:
        _prog_cache[key] = _build(causal, reps)
    return _prog_cache[key]


def _bf16():
    import ml_dtypes
    return ml_dtypes.bfloat16


def _host_prep(query, key, value, mask, w_q, w_k, w_v, w_o):
    """Build the 8 per-core input maps. Returns (in_maps, causal)."""
    f32 = np.float32
    tril = np.tril(np.ones((S, S), dtype=mask.dtype))
    causal = all(np.array_equal(mask[b], tril) for b in range(B))

    kp = np.arange(128)[:, None].astype(f32)
    madd_d = np.where(np.arange(128)[None, :] > kp, f32(NEG), f32(0.0))
    u = np.arange(1024)[None, :].astype(f32)
    madd_tw = np.where(u < kp + 512, f32(NEG), f32(0.0)).astype(f32)

    in_maps = []
    for c in range(N_CORES):
        b = c // 2
        hs = (c % 2) * HPC
        r0, r1 = hs * DK, (hs + HPC) * DK
        m = {
            "xqT": np.ascontiguousarray(query[b].T.astype(f32)),
            "xkT": np.ascontiguousarray(key[b].T.astype(f32)),
            "xvT": np.ascontiguousarray(value[b].T.astype(f32)),
            "wqT": np.ascontiguousarray(w_q[r0:r1, :].T.astype(f32)),
            "wkT": np.ascontiguousarray(w_k[r0:r1, :].T.astype(f32)),
            "wvT": np.ascontiguousarray(w_v[r0:r1, :].T.astype(f32)),
            "woT": np.ascontiguousarray(w_o[:, r0:r1].T).astype(_bf16()),
            "madd_d": np.ascontiguousarray(madd_d.astype(f32)),
            "madd_tw": np.ascontiguousarray(madd_tw.astype(f32)),
        }
        if not causal:
            mf = np.where(mask[b] == 0, f32(NEG), f32(0.0)).astype(f32)
            m["maddf"] = np.ascontiguousarray(mf)
            m["maddfT"] = np.ascontiguousarray(mf.T)
        in_maps.append(m)
    return in_maps, causal


def _gather(results, b_o):
    out = np.zeros((B, S, D), dtype=np.float32)
    attn_full = np.empty((B, H, S, S), dtype=np.float32)
    for c in range(N_CORES):
        b = c // 2
        hs = (c % 2) * HPC
        out[b] += results[c]["o"]
        attn_full[b, hs:hs + HPC] = results[c]["attn"]
    out += np.asarray(b_o, dtype=np.float32)[None, None, :]
    return out, attn_full


def kernel(query, key, value, mask, w_q, w_k, w_v, w_o, b_o):
    from concourse.bass_utils import run_bass_kernel_spmd
    in_maps, causal = _host_prep(query, key, value, mask, w_q, w_k, w_v, w_o)
    nc = _get_prog(causal)
    res = run_bass_kernel_spmd(nc, in_maps, core_ids=list(range(N_CORES)))
    return _gather(res.results, b_o)
